# revision 16
# baseline (speedup 1.0000x reference)
"""Trainium2 Bass kernel for nn_Attention_28819230556655.

Gated-adapter causal attention (LLaMA-adapter style). Sharding: batch x
head-group over 8 NeuronCores — core c owns batch c//4 and global heads
[(c%4)*8, (c%4)*8+8). All matmul operands are bf16 (fast weight load + full
streaming rate); accumulation is fp32 in PSUM.

Per-core dataflow (no DRAM intermediates, no on-device transposes):
- host pre-transposes x to x^T [D, S] bf16 and rope tables to [64, S].
- projections are weight-stationary producing q^T/k^T [hd, tok] directly;
  wq/wk columns are permuted (even pairs first) so RoPE works on partition
  halves; v is x^T-stationary producing v [tok, hd] (the PV stationary).
- scores are computed transposed (S^T = k^T.T @ q^T) so exp output P^T feeds
  the PV matmul with no transposes; row sums come from an all-ones stationary
  matmul accumulated alongside PV; normalization happens on the tiny
  attn^T [hd, q] result.
- output projection contracts attn^T (SBUF-resident) against wo.
Host sums the 4 partial outputs per batch.
"""
import math
import numpy as np

import concourse.bass as bass
import concourse.bacc as bacc
import concourse.mybir as mybir
import concourse.tile as tile
from concourse.bass_utils import run_bass_kernel_spmd

F32 = mybir.dt.float32
BF16 = mybir.dt.bfloat16

# ---- problem constants ----
B, S, D, H = 2, 2048, 4096, 32
HD, HALF, AL = 128, 64, 10
NCORES = 8
HPC = 8            # heads per core
HPP = 2            # heads per pass
NPASS = HPC // HPP
TC = 512           # token chunk
ALP = 16           # padded adapter rows
SCALE = 1.0 / math.sqrt(HD)
MASKV = -12000.0   # pre-scale additive mask -> exp == 0


def build_nc(s=S, d=D, npass=NPASS):
    """SPMD per-core program. s/d/npass overridable for small tests."""
    kb_n = d // 128
    ntc = s // TC if s >= TC else 1
    tc = min(TC, s)
    nkt = s // 128

    nc = bacc.Bacc()
    xT_in = nc.declare_dram_parameter("xT", [kb_n, 128, s], BF16, isOutput=False)
    wqk_in = nc.declare_dram_parameter("wqk", [npass, kb_n, 128, 512], BF16,
                                       isOutput=False)
    wv_in = nc.declare_dram_parameter("wv", [npass, kb_n, 128, 256], BF16,
                                      isOutput=False)
    wo_in = nc.declare_dram_parameter("wo", [128, npass * HPP, d], BF16,
                                      isOutput=False)
    cosT_in = nc.declare_dram_parameter("cosT", [HALF, s], F32, isOutput=False)
    sinT_in = nc.declare_dram_parameter("sinT", [HALF, s], F32, isOutput=False)
    akT_in = nc.declare_dram_parameter("akT", [128, npass * HPP, AL], BF16,
                                       isOutput=False)
    av_in = nc.declare_dram_parameter("av", [AL, npass * HPP, 128], BF16,
                                      isOutput=False)
    out_d = nc.declare_dram_parameter("out", [s, d], F32, isOutput=True)

    hpc = npass * HPP

    with tile.TileContext(nc) as tc_:
        with tc_.tile_pool(name="const", bufs=1) as cpool:
            # all-ones stationary for partition rowsums
            ones_f = cpool.tile([128, 128], F32)
            nc.vector.memset(ones_f, 1.0)
            ones_bf = cpool.tile([128, 128], BF16)
            nc.vector.tensor_copy(ones_bf, ones_f)
            # transposed causal diagonal mask: [k-part, q-free], fill where k>q
            diagT = cpool.tile([128, 128], F32)
            nc.vector.memset(diagT, 0.0)
            nc.gpsimd.affine_select(
                out=diagT, in_=diagT, compare_op=mybir.AluOpType.is_ge,
                fill=MASKV, base=0, pattern=[[1, 128]], channel_multiplier=-1,
            )
            # host-precomputed adapter projections
            akT_sb = cpool.tile([128, hpc, AL], BF16)
            nc.gpsimd.dma_start(out=akT_sb, in_=akT_in[:, 0:hpc, :])
            av_all = cpool.tile([ALP, hpc, 128], BF16)
            nc.gpsimd.dma_start(out=av_all[0:AL], in_=av_in[:, 0:hpc, :])
            # rope tables [64, s]
            cosT = cpool.tile([HALF, s], F32)
            sinT = cpool.tile([HALF, s], F32)
            nc.gpsimd.dma_start(out=cosT, in_=cosT_in[:, :])
            nc.gpsimd.dma_start(out=sinT, in_=sinT_in[:, :])
            # persistent attention output, [hd, head, tok]
            attnT = cpool.tile([128, hpc, s], BF16)

            xT_r = xT_in.rearrange("kb p t -> p kb t")

            with (
                tc_.tile_pool(name="w", bufs=1) as wpool,
                tc_.tile_pool(name="xio", bufs=2) as xio,
                tc_.tile_pool(name="qk", bufs=2) as qkpool,
                tc_.tile_pool(name="ad", bufs=2) as adpool,
                tc_.tile_pool(name="pt", bufs=3) as ptpool,
                tc_.tile_pool(name="nrm", bufs=4) as nrmpool,
                tc_.tile_pool(name="pj", bufs=2, space="PSUM") as pj,
                tc_.tile_pool(name="sc", bufs=2, space="PSUM") as scp,
                tc_.tile_pool(name="acc", bufs=4, space="PSUM") as accp,
            ):
                for p_ in range(npass):
                    wqk_sb = wpool.tile([128, kb_n, 512], BF16, tag="wqk",
                                        name=f"wqk{p_}")
                    wv_sb = wpool.tile([128, kb_n, 256], BF16, tag="wv",
                                       bufs=2, name=f"wv{p_}")
                    nc.sync.dma_start(
                        out=wqk_sb, in_=wqk_in[p_].rearrange("kb p c -> p kb c"))
                    nc.sync.dma_start(
                        out=wv_sb, in_=wv_in[p_].rearrange("kb p c -> p kb c"))

                    # ---- per-pass persistent q/k/v ----
                    qT = [qkpool.tile([128, s], BF16, tag="qT", name=f"qT{i}")
                          for i in range(HPP)]
                    kT = [qkpool.tile([128, s], BF16, tag="kT", name=f"kT{i}")
                          for i in range(HPP)]
                    v_sb = qkpool.tile([128, nkt, 256], BF16, tag="vsb")

                    for tci in range(ntc):
                        t0 = tci * tc
                        kbq = max(kb_n // 4, 1)
                        xcq = []
                        for qi in range(kb_n // kbq):
                            xq_ = xio.tile([128, kbq, tc], BF16, tag="xc",
                                           bufs=4, name=f"xcq{qi}")
                            nc.gpsimd.dma_start(
                                out=xq_,
                                in_=xT_r[:, qi * kbq:(qi + 1) * kbq, t0:t0 + tc])
                            xcq.append(xq_)

                        def xcs(kb):
                            return xcq[kb // kbq][:, kb % kbq, :]

                        # ---- q/k projections + rope ----
                        for (hh, c0, dst) in ((0, 0, qT[0]), (1, 128, qT[1]),
                                              (0, 256, kT[0]), (1, 384, kT[1])):
                            pq = pj.tile([128, tc], F32, tag="pj")
                            for kb in range(kb_n):
                                nc.tensor.matmul(
                                    pq, wqk_sb[:, kb, c0:c0 + 128], xcs(kb),
                                    start=(kb == 0), stop=(kb == kb_n - 1))
                            ev, od = pq[0:HALF, :], pq[HALF:128, :]
                            ct = cosT[:, t0:t0 + tc]
                            st = sinT[:, t0:t0 + tc]
                            ec = nrmpool.tile([HALF, tc], BF16, tag="rt", bufs=2)
                            os_ = nrmpool.tile([HALF, tc], BF16, tag="rt", bufs=2)
                            es = nrmpool.tile([HALF, tc], BF16, tag="rt", bufs=2)
                            oc = nrmpool.tile([HALF, tc], BF16, tag="rt", bufs=2)
                            nc.vector.tensor_mul(ec, ev, ct)
                            nc.vector.tensor_mul(os_, od, st)
                            nc.vector.tensor_mul(es, ev, st)
                            nc.vector.tensor_mul(oc, od, ct)
                            nc.vector.tensor_sub(dst[0:HALF, t0:t0 + tc], ec, os_)
                            nc.vector.tensor_add(dst[HALF:128, t0:t0 + tc], es, oc)

                        # ---- v projection ([tok, hd] orientation) ----
                        for sti in range(tc // 128):
                            pv = pj.tile([128, 256], F32, tag="pj")
                            xsl = slice(sti * 128, sti * 128 + 128)
                            for kb in range(kb_n):
                                nc.tensor.matmul(
                                    pv, xcs(kb)[:, xsl], wv_sb[:, kb, :],
                                    start=(kb == 0), stop=(kb == kb_n - 1))
                            kt_i = (t0 // 128) + sti
                            nc.scalar.copy(v_sb[:, kt_i, :], pv)

                        # ---- attention for this q chunk (heads interleaved) --
                        # adapter branch first for both heads (psums freed fast)
                        t2 = [None, None]
                        for hh in range(HPP):
                            g = p_ * HPP + hh
                            saT = scp.tile([128, tc], F32, tag="sc")
                            nc.tensor.matmul(saT[0:AL, :], akT_sb[:, g, :],
                                             qT[hh][:, t0:t0 + tc],
                                             start=True, stop=True)
                            paT = ptpool.tile([ALP, tc], BF16, tag="pa", bufs=2)
                            nc.scalar.activation(
                                paT[0:AL, :], saT[0:AL, :],
                                mybir.ActivationFunctionType.Exp, scale=SCALE)
                            ra_ps = accp.tile([128, tc], F32, tag="acc")
                            nc.tensor.matmul(ra_ps, ones_bf[0:AL, :],
                                             paT[0:AL, :], start=True, stop=True)
                            aa_ps = accp.tile([128, tc], F32, tag="acc")
                            nc.tensor.matmul(aa_ps, av_all[0:AL, g, :],
                                             paT[0:AL, :], start=True, stop=True)
                            ra_sb = nrmpool.tile([128, tc], F32, tag="rcp",
                                                 bufs=2)
                            nc.scalar.copy(ra_sb, ra_ps)
                            aa_sb = nrmpool.tile([128, tc], BF16, tag="asb",
                                                 bufs=2)
                            nc.scalar.copy(aa_sb, aa_ps)
                            rainv = nrmpool.tile([128, tc], F32, tag="rcp",
                                                 bufs=2)
                            nc.vector.reciprocal(rainv, ra_sb)
                            t2[hh] = nrmpool.tile([128, tc], BF16, tag="tmp",
                                                  bufs=4, name=f"t2_{hh}")
                            nc.vector.tensor_mul(t2[hh], aa_sb, rainv)

                        # main causal attention, both heads block-interleaved
                        at_ps = [accp.tile([128, tc], F32, tag="acc",
                                           name=f"at{i}") for i in range(HPP)]
                        r_ps = [accp.tile([128, tc], F32, tag="acc",
                                          name=f"r{i}") for i in range(HPP)]
                        nkb = (t0 + tc) // 128
                        for kb in range(nkb):
                            lo = max(0, kb * 128 - t0)
                            for hh in range(HPP):
                                sT = scp.tile([128, tc], F32, tag="sc")
                                nc.tensor.matmul(
                                    sT[:, lo:tc],
                                    kT[hh][:, kb * 128:kb * 128 + 128],
                                    qT[hh][:, t0 + lo:t0 + tc],
                                    start=True, stop=True)
                                if kb * 128 >= t0:  # diagonal block
                                    nc.vector.tensor_add(
                                        sT[:, lo:lo + 128], sT[:, lo:lo + 128],
                                        diagT)
                                pT = ptpool.tile([128, tc], BF16, tag="pt",
                                                 bufs=4)
                                nc.scalar.activation(
                                    pT[:, lo:tc], sT[:, lo:tc],
                                    mybir.ActivationFunctionType.Exp, scale=SCALE)
                                nc.tensor.matmul(r_ps[hh][:, lo:tc], ones_bf,
                                                 pT[:, lo:tc],
                                                 start=(kb == 0),
                                                 stop=(kb == nkb - 1))
                                nc.tensor.matmul(at_ps[hh][:, lo:tc],
                                                 v_sb[:, kb,
                                                      hh * 128:hh * 128 + 128],
                                                 pT[:, lo:tc],
                                                 start=(kb == 0),
                                                 stop=(kb == nkb - 1))
                        for hh in range(HPP):
                            g = p_ * HPP + hh
                            r_sb = nrmpool.tile([128, tc], F32, tag="rcp",
                                                bufs=2)
                            nc.scalar.copy(r_sb, r_ps[hh])
                            at_sb = nrmpool.tile([128, tc], BF16, tag="asb",
                                                 bufs=2)
                            nc.scalar.copy(at_sb, at_ps[hh])
                            rinv = nrmpool.tile([128, tc], F32, tag="rcp",
                                                bufs=2)
                            nc.vector.reciprocal(rinv, r_sb)
                            t1 = nrmpool.tile([128, tc], BF16, tag="tmp", bufs=4)
                            nc.vector.tensor_mul(t1, at_sb, rinv)
                            nc.vector.tensor_add(attnT[:, g, t0:t0 + tc], t1,
                                                 t2[hh])

            # ---- output projection ----
            with (
                tc_.tile_pool(name="wo", bufs=1) as wop,
                tc_.tile_pool(name="ob", bufs=3) as obuf,
                tc_.tile_pool(name="ops", bufs=2, space="PSUM") as wps,
            ):
                wo_sb = wop.tile([128, hpc, d], BF16)
                nc.sync.dma_start(out=wo_sb, in_=wo_in[:, 0:hpc, :])
                for tt in range(nkt):
                    tsl = slice(tt * 128, tt * 128 + 128)
                    for ocs in range(d // 512):
                        op_ = wps.tile([128, 512], F32, tag="wo")
                        osl = slice(ocs * 512, ocs * 512 + 512)
                        for h in range(hpc):
                            nc.tensor.matmul(op_, attnT[:, h, tsl],
                                             wo_sb[:, h, osl],
                                             start=(h == 0), stop=(h == hpc - 1))
                        ost = obuf.tile([128, 512], F32, tag="ost")
                        nc.vector.tensor_copy(ost, op_)
                        nc.sync.dma_start(out=out_d[tsl, osl], in_=ost)
    nc.finalize()
    return nc


_PERM = np.concatenate([np.arange(0, HD, 2), np.arange(1, HD, 2)])


def _host_inputs(core, x, cos, sin, wq, wk, wv, wo, gate, adapter, s=S, d=D,
                 npass=NPASS, xT_cache=None):
    """Per-core input map. core -> batch core//4, heads [(core%4)*8, +8)."""
    import ml_dtypes
    bf = ml_dtypes.bfloat16
    kb_n = d // 128
    hpc = npass * HPP
    bi = core // (NCORES // B)
    g0 = (core % (NCORES // B)) * hpc

    if xT_cache is not None and bi in xT_cache:
        xT = xT_cache[bi]
    else:
        xT = np.ascontiguousarray(x[bi].T.astype(bf)).reshape(kb_n, 128, s)
        if xT_cache is not None:
            xT_cache[bi] = xT

    wqk = np.empty((npass, kb_n, 128, 512), bf)
    wvs = np.empty((npass, kb_n, 128, 256), bf)
    wos = np.empty((128, hpc, d), bf)
    for lh in range(hpc):
        g = g0 + lh
        p_, hh = lh // HPP, lh % HPP
        qrows = wq[g * 128:(g + 1) * 128][_PERM].T.reshape(kb_n, 128, 128)
        krows = wk[g * 128:(g + 1) * 128][_PERM].T.reshape(kb_n, 128, 128)
        vrows = wv[g * 128:(g + 1) * 128].T.reshape(kb_n, 128, 128)
        wqk[p_, :, :, hh * 128:(hh + 1) * 128] = qrows.astype(bf)
        wqk[p_, :, :, 256 + hh * 128:256 + (hh + 1) * 128] = krows.astype(bf)
        wvs[p_, :, :, hh * 128:(hh + 1) * 128] = vrows.astype(bf)
        wos[:, lh, :] = wo[:, g * 128:(g + 1) * 128].T.astype(bf)
    # rope tables transposed; pair i lives at partitions (i, 64+i)
    cosT = np.ascontiguousarray(cos[:s].T).astype(np.float32)
    sinT = np.ascontiguousarray(sin[:s].T).astype(np.float32)
    # adapter projections on host (tiny): a_k^T (perm rows) and tanh(g)*a_v
    hsl = slice(g0 * 128, (g0 + hpc) * 128)
    ak = adapter[bi] @ wk[hsl].T          # [AL, hpc*128]
    av = adapter[bi] @ wv[hsl].T
    akT = np.empty((128, hpc, AL), bf)
    avs = np.empty((AL, hpc, 128), bf)
    for lh in range(hpc):
        akT[:, lh, :] = ak[:, lh * 128:(lh + 1) * 128].T[_PERM].astype(bf)
        g_ = math.tanh(float(gate[0, g0 + lh, 0, 0]))
        avs[:, lh, :] = (av[:, lh * 128:(lh + 1) * 128] * g_).astype(bf)
    return {
        "xT": xT, "wqk": wqk, "wv": wvs, "wo": wos,
        "cosT": cosT, "sinT": sinT, "akT": akT, "av": avs,
    }


def _numpy_reference(x, mask, cos, sin, wq, wk, wv, wo, gate, adapter):
    """Fallback (and general-mask) path in fp32 numpy."""
    bsz, seqlen, dm = x.shape
    h = wq.shape[0] // HD
    sc = 1.0 / math.sqrt(HD)

    def rope(t):
        tr = t.reshape(*t.shape[:-1], HD // 2, 2)
        t0, t1 = tr[..., 0], tr[..., 1]
        c = cos[None, :, None, :]
        s_ = sin[None, :, None, :]
        r0 = t0 * c - t1 * s_
        r1 = t0 * s_ + t1 * c
        return np.stack([r0, r1], axis=-1).reshape(t.shape)

    xq = (x @ wq.T).reshape(bsz, seqlen, h, HD)
    xk = (x @ wk.T).reshape(bsz, seqlen, h, HD)
    xv = (x @ wv.T).reshape(bsz, seqlen, h, HD)
    q = rope(xq).transpose(0, 2, 1, 3)
    k = rope(xk).transpose(0, 2, 1, 3)
    v = xv.transpose(0, 2, 1, 3)
    sc_ = np.einsum("bhqd,bhkd->bhqk", q, k) * sc + mask
    sc_ = sc_ - sc_.max(-1, keepdims=True)
    e = np.exp(sc_)
    p = e / e.sum(-1, keepdims=True)
    out = np.einsum("bhqk,bhkd->bhqd", p, v)
    al = adapter.shape[1]
    av = (adapter @ wv.T).reshape(bsz, al, h, HD).transpose(0, 2, 1, 3)
    ak = (adapter @ wk.T).reshape(bsz, al, h, HD).transpose(0, 2, 1, 3)
    asc = np.einsum("bhqd,bhkd->bhqk", q, ak) * sc
    asc = asc - asc.max(-1, keepdims=True)
    ae = np.exp(asc)
    ap = np.tanh(gate) * ae / ae.sum(-1, keepdims=True)
    out = out + np.einsum("bhqk,bhkd->bhqd", ap, av)
    out = out.transpose(0, 2, 1, 3).reshape(bsz, seqlen, -1)
    return (out @ wo.T).astype(np.float32)


_NC_CACHE = {}


def kernel(x, mask, cos, sin, wq, wk, wv, wo, gate, adapter, start_pos):
    x = np.asarray(x, np.float32)
    mask = np.asarray(mask, np.float32)
    cos = np.asarray(cos, np.float32)
    sin = np.asarray(sin, np.float32)
    wq = np.asarray(wq, np.float32)
    wk = np.asarray(wk, np.float32)
    wv = np.asarray(wv, np.float32)
    wo = np.asarray(wo, np.float32)
    gate = np.asarray(gate, np.float32)
    adapter = np.asarray(adapter, np.float32)

    causal = np.triu(np.full((S, S), -1e9, np.float32), 1)[None, None]
    if (x.shape != (B, S, D) or int(start_pos) != 0
            or not np.array_equal(mask, causal)):
        return _numpy_reference(x, mask, cos, sin, wq, wk, wv, wo, gate, adapter)

    if "nc" not in _NC_CACHE:
        _NC_CACHE["nc"] = build_nc()
    nc = _NC_CACHE["nc"]
    xT_cache = {}
    in_maps = [
        _host_inputs(c, x, cos, sin, wq, wk, wv, wo, gate, adapter,
                     xT_cache=xT_cache)
        for c in range(NCORES)
    ]
    res = run_bass_kernel_spmd(nc, in_maps, list(range(NCORES)))
    gpb = NCORES // B
    out = np.empty((B, S, D), np.float64)
    for bi in range(B):
        acc = res.results[bi * gpb]["out"].astype(np.float64)
        for c in range(bi * gpb + 1, (bi + 1) * gpb):
            acc += res.results[c]["out"]
        out[bi] = acc
    return out.astype(np.float32)


# revision 17
# speedup vs baseline: 1.2603x; 1.2603x over previous
"""Trainium2 Bass kernel for nn_Attention_28819230556655.

Gated-adapter causal attention (LLaMA-adapter style). Sharding: batch x
head-group over 8 NeuronCores — core c owns batch c//4 and global heads
[(c%4)*8, (c%4)*8+8). All matmul operands are bf16 (fast weight load + full
streaming rate); accumulation is fp32 in PSUM.

Per-core dataflow (no DRAM intermediates, no on-device transposes):
- host pre-transposes x to x^T [D, S] bf16 and rope tables to [64, S].
- projections are weight-stationary producing q^T/k^T [hd, tok] directly;
  wq/wk columns are permuted (even pairs first) so RoPE works on partition
  halves; v is x^T-stationary producing v [tok, hd] (the PV stationary).
- scores are computed transposed (S^T = k^T.T @ q^T) so exp output P^T feeds
  the PV matmul with no transposes; row sums come from an all-ones stationary
  matmul accumulated alongside PV; normalization happens on the tiny
  attn^T [hd, q] result.
- output projection contracts attn^T (SBUF-resident) against wo.
Host sums the 4 partial outputs per batch.
"""
import math
import numpy as np

import concourse.bass as bass
import concourse.bacc as bacc
import concourse.mybir as mybir
import concourse.tile as tile
from concourse.bass_utils import run_bass_kernel_spmd

F32 = mybir.dt.float32
BF16 = mybir.dt.bfloat16

# ---- problem constants ----
B, S, D, H = 2, 2048, 4096, 32
HD, HALF, AL = 128, 64, 10
NCORES = 8
HPC = 8            # heads per core
HPP = 2            # heads per pass
NPASS = HPC // HPP
TC = 512           # token chunk
ALP = 16           # padded adapter rows
SCALE = 1.0 / math.sqrt(HD)
MASKV = -12000.0   # pre-scale additive mask -> exp == 0


def build_nc(s=S, d=D, npass=NPASS):
    """SPMD per-core program. s/d/npass overridable for small tests."""
    kb_n = d // 128
    ntc = s // TC if s >= TC else 1
    tc = min(TC, s)
    nkt = s // 128

    nc = bacc.Bacc()
    xT_in = nc.declare_dram_parameter("xT", [kb_n, 128, s], BF16, isOutput=False)
    wqk_in = nc.declare_dram_parameter("wqk", [npass, kb_n, 128, 512], BF16,
                                       isOutput=False)
    wv_in = nc.declare_dram_parameter("wv", [npass, kb_n, 128, 256], BF16,
                                      isOutput=False)
    wo_in = nc.declare_dram_parameter("wo", [128, npass * HPP, d], BF16,
                                      isOutput=False)
    cosT_in = nc.declare_dram_parameter("cosT", [HALF, s], F32, isOutput=False)
    sinT_in = nc.declare_dram_parameter("sinT", [HALF, s], F32, isOutput=False)
    akT_in = nc.declare_dram_parameter("akT", [128, npass * HPP, AL], BF16,
                                       isOutput=False)
    av_in = nc.declare_dram_parameter("av", [AL, npass * HPP, 128], BF16,
                                      isOutput=False)
    out_d = nc.declare_dram_parameter("out", [s, d], F32, isOutput=True)

    hpc = npass * HPP

    with tile.TileContext(nc) as tc_:
        with tc_.tile_pool(name="const", bufs=1) as cpool:
            # all-ones stationary for partition rowsums
            ones_f = cpool.tile([128, 128], F32)
            nc.vector.memset(ones_f, 1.0)
            ones_bf = cpool.tile([128, 128], BF16)
            nc.vector.tensor_copy(ones_bf, ones_f)
            # transposed causal diagonal mask: [k-part, q-free], fill where k>q
            diagT = cpool.tile([128, 128], F32)
            nc.vector.memset(diagT, 0.0)
            nc.gpsimd.affine_select(
                out=diagT, in_=diagT, compare_op=mybir.AluOpType.is_ge,
                fill=MASKV, base=0, pattern=[[1, 128]], channel_multiplier=-1,
            )
            # host-precomputed adapter projections
            akT_sb = cpool.tile([128, hpc, AL], BF16)
            nc.sync.dma_start(out=akT_sb, in_=akT_in[:, 0:hpc, :])
            av_all = cpool.tile([ALP, hpc, 128], BF16)
            nc.sync.dma_start(out=av_all[0:AL], in_=av_in[:, 0:hpc, :])
            # rope tables [64, s]
            cosT = cpool.tile([HALF, s], F32)
            sinT = cpool.tile([HALF, s], F32)
            nc.sync.dma_start(out=cosT, in_=cosT_in[:, :])
            nc.sync.dma_start(out=sinT, in_=sinT_in[:, :])
            # persistent attention output, [hd, head, tok]
            attnT = cpool.tile([128, hpc, s], BF16)

            xT_r = xT_in.rearrange("kb p t -> p kb t")

            with (
                tc_.tile_pool(name="w", bufs=1) as wpool,
                tc_.tile_pool(name="xio", bufs=2) as xio,
                tc_.tile_pool(name="qk", bufs=2) as qkpool,
                tc_.tile_pool(name="ad", bufs=2) as adpool,
                tc_.tile_pool(name="pt", bufs=3) as ptpool,
                tc_.tile_pool(name="nrm", bufs=4) as nrmpool,
                tc_.tile_pool(name="pj", bufs=2, space="PSUM") as pj,
                tc_.tile_pool(name="sc", bufs=2, space="PSUM") as scp,
                tc_.tile_pool(name="acc", bufs=4, space="PSUM") as accp,
            ):
                for p_ in range(npass):
                    wqk_sb = wpool.tile([128, kb_n, 512], BF16, tag="wqk",
                                        name=f"wqk{p_}")
                    wv_sb = wpool.tile([128, kb_n, 256], BF16, tag="wv",
                                       bufs=2, name=f"wv{p_}")
                    nc.sync.dma_start(
                        out=wqk_sb, in_=wqk_in[p_].rearrange("kb p c -> p kb c"))
                    nc.sync.dma_start(
                        out=wv_sb, in_=wv_in[p_].rearrange("kb p c -> p kb c"))

                    # ---- per-pass persistent q/k/v ----
                    qT = [qkpool.tile([128, s], BF16, tag="qT", name=f"qT{i}")
                          for i in range(HPP)]
                    kT = [qkpool.tile([128, s], BF16, tag="kT", name=f"kT{i}")
                          for i in range(HPP)]
                    v_sb = [qkpool.tile([128, nkt, 128], BF16, tag="vsb",
                                        name=f"vsb{i}") for i in range(HPP)]

                    for tci in range(ntc):
                        t0 = tci * tc
                        kbq = max(kb_n // 4, 1)
                        xcq = []
                        for qi in range(kb_n // kbq):
                            xq_ = xio.tile([128, kbq, tc], BF16, tag="xc",
                                           bufs=6, name=f"xcq{qi}")
                            nc.sync.dma_start(
                                out=xq_,
                                in_=xT_r[:, qi * kbq:(qi + 1) * kbq, t0:t0 + tc])
                            xcq.append(xq_)

                        def xcs(kb):
                            return xcq[kb // kbq][:, kb % kbq, :]

                        # ---- q/k projections + rope ----
                        for (hh, c0, dst) in ((0, 0, qT[0]), (1, 128, qT[1]),
                                              (0, 256, kT[0]), (1, 384, kT[1])):
                            pq = pj.tile([128, tc], F32, tag="pj")
                            for kb in range(kb_n):
                                nc.tensor.matmul(
                                    pq, wqk_sb[:, kb, c0:c0 + 128], xcs(kb),
                                    start=(kb == 0), stop=(kb == kb_n - 1))
                            ev, od = pq[0:HALF, :], pq[HALF:128, :]
                            ct = cosT[:, t0:t0 + tc]
                            st = sinT[:, t0:t0 + tc]
                            ec = nrmpool.tile([HALF, tc], BF16, tag="rt", bufs=2)
                            os_ = nrmpool.tile([HALF, tc], BF16, tag="rt", bufs=2)
                            es = nrmpool.tile([HALF, tc], BF16, tag="rt", bufs=2)
                            oc = nrmpool.tile([HALF, tc], BF16, tag="rt", bufs=2)
                            nc.vector.tensor_mul(ec, ev, ct)
                            nc.vector.tensor_mul(os_, od, st)
                            nc.vector.tensor_mul(es, ev, st)
                            nc.vector.tensor_mul(oc, od, ct)
                            nc.vector.tensor_sub(dst[0:HALF, t0:t0 + tc], ec, os_)
                            nc.vector.tensor_add(dst[HALF:128, t0:t0 + tc], es, oc)

                        # ---- v projection ([tok, hd] orientation) ----
                        for sti in range(tc // 128):
                            pv = pj.tile([128, 256], F32, tag="pj")
                            xsl = slice(sti * 128, sti * 128 + 128)
                            for kb in range(kb_n):
                                nc.tensor.matmul(
                                    pv, xcs(kb)[:, xsl], wv_sb[:, kb, :],
                                    start=(kb == 0), stop=(kb == kb_n - 1))
                            kt_i = (t0 // 128) + sti
                            nc.scalar.copy(v_sb[0][:, kt_i, :], pv[:, 0:128])
                            nc.scalar.copy(v_sb[1][:, kt_i, :], pv[:, 128:256])

                        # ---- attention for this q chunk ----
                        for hh in range(HPP):
                            g = p_ * HPP + hh
                            # adapter branch first (frees its psum early)
                            saT = scp.tile([128, tc], F32, tag="sc")
                            nc.tensor.matmul(saT[0:AL, :], akT_sb[:, g, :],
                                             qT[hh][:, t0:t0 + tc],
                                             start=True, stop=True)
                            paT = ptpool.tile([ALP, tc], BF16, tag="pa", bufs=2)
                            nc.scalar.activation(
                                paT[0:AL, :], saT[0:AL, :],
                                mybir.ActivationFunctionType.Exp, scale=SCALE)
                            ra_ps = accp.tile([128, tc], F32, tag="acc")
                            nc.tensor.matmul(ra_ps, ones_bf[0:AL, :],
                                             paT[0:AL, :], start=True, stop=True)
                            aa_ps = accp.tile([128, tc], F32, tag="acc")
                            nc.tensor.matmul(aa_ps, av_all[0:AL, g, :],
                                             paT[0:AL, :], start=True, stop=True)
                            # free both psums fast via ACT copies, then
                            # reciprocal+mult on DVE from SBUF
                            ra_sb = nrmpool.tile([128, tc], F32, tag="rcp",
                                                 bufs=2)
                            nc.scalar.copy(ra_sb, ra_ps)
                            aa_sb = nrmpool.tile([128, tc], BF16, tag="asb",
                                                 bufs=2)
                            nc.scalar.copy(aa_sb, aa_ps)
                            rainv = nrmpool.tile([128, tc], F32, tag="rcp",
                                                 bufs=2)
                            nc.vector.reciprocal(rainv, ra_sb)
                            t2 = nrmpool.tile([128, tc], BF16, tag="tmp",
                                              bufs=2)
                            nc.vector.tensor_mul(t2, aa_sb, rainv)

                            # main causal attention
                            at_ps = accp.tile([128, tc], F32, tag="acc")
                            r_ps = accp.tile([128, tc], F32, tag="acc")
                            nkb = (t0 + tc) // 128
                            for kb in range(nkb):
                                lo = max(0, kb * 128 - t0)
                                sT = scp.tile([128, tc], F32, tag="sc")
                                nc.tensor.matmul(
                                    sT[:, lo:tc], kT[hh][:, kb * 128:kb * 128 + 128],
                                    qT[hh][:, t0 + lo:t0 + tc],
                                    start=True, stop=True)
                                if kb * 128 >= t0:  # diagonal block
                                    nc.vector.tensor_add(
                                        sT[:, lo:lo + 128], sT[:, lo:lo + 128],
                                        diagT)
                                pT = ptpool.tile([128, tc], BF16, tag="pt")
                                nc.scalar.activation(
                                    pT[:, lo:tc], sT[:, lo:tc],
                                    mybir.ActivationFunctionType.Exp, scale=SCALE)
                                nc.tensor.matmul(r_ps[:, lo:tc], ones_bf,
                                                 pT[:, lo:tc],
                                                 start=(kb == 0), stop=(kb == nkb - 1))
                                nc.tensor.matmul(at_ps[:, lo:tc],
                                                 v_sb[hh][:, kb, :], pT[:, lo:tc],
                                                 start=(kb == 0), stop=(kb == nkb - 1))
                            r_sb = nrmpool.tile([128, tc], F32, tag="rcp",
                                                bufs=2)
                            nc.scalar.copy(r_sb, r_ps)
                            at_sb = nrmpool.tile([128, tc], BF16, tag="asb",
                                                 bufs=2)
                            nc.scalar.copy(at_sb, at_ps)
                            rinv = nrmpool.tile([128, tc], F32, tag="rcp", bufs=2)
                            nc.vector.reciprocal(rinv, r_sb)
                            t1 = nrmpool.tile([128, tc], BF16, tag="tmp", bufs=2)
                            nc.vector.tensor_mul(t1, at_sb, rinv)
                            nc.vector.tensor_add(attnT[:, g, t0:t0 + tc], t1, t2)

            # ---- output projection ----
            with (
                tc_.tile_pool(name="wo", bufs=1) as wop,
                tc_.tile_pool(name="ob", bufs=3) as obuf,
                tc_.tile_pool(name="ops", bufs=2, space="PSUM") as wps,
            ):
                wo_sb = wop.tile([128, hpc, d], BF16)
                nc.sync.dma_start(out=wo_sb, in_=wo_in[:, 0:hpc, :])
                for tt in range(nkt):
                    tsl = slice(tt * 128, tt * 128 + 128)
                    for ocs in range(d // 512):
                        op_ = wps.tile([128, 512], F32, tag="wo")
                        osl = slice(ocs * 512, ocs * 512 + 512)
                        for h in range(hpc):
                            nc.tensor.matmul(op_, attnT[:, h, tsl],
                                             wo_sb[:, h, osl],
                                             start=(h == 0), stop=(h == hpc - 1))
                        ost = obuf.tile([128, 512], F32, tag="ost")
                        nc.vector.tensor_copy(ost, op_)
                        nc.sync.dma_start(out=out_d[tsl, osl], in_=ost)
    nc.finalize()
    return nc


_PERM = np.concatenate([np.arange(0, HD, 2), np.arange(1, HD, 2)])


def _host_inputs(core, x, cos, sin, wq, wk, wv, wo, gate, adapter, s=S, d=D,
                 npass=NPASS, xT_cache=None):
    """Per-core input map. core -> batch core//4, heads [(core%4)*8, +8)."""
    import ml_dtypes
    bf = ml_dtypes.bfloat16
    kb_n = d // 128
    hpc = npass * HPP
    bi = core // (NCORES // B)
    g0 = (core % (NCORES // B)) * hpc

    if xT_cache is not None and bi in xT_cache:
        xT = xT_cache[bi]
    else:
        xT = np.ascontiguousarray(x[bi].T.astype(bf)).reshape(kb_n, 128, s)
        if xT_cache is not None:
            xT_cache[bi] = xT

    wqk = np.empty((npass, kb_n, 128, 512), bf)
    wvs = np.empty((npass, kb_n, 128, 256), bf)
    wos = np.empty((128, hpc, d), bf)
    for lh in range(hpc):
        g = g0 + lh
        p_, hh = lh // HPP, lh % HPP
        qrows = wq[g * 128:(g + 1) * 128][_PERM].T.reshape(kb_n, 128, 128)
        krows = wk[g * 128:(g + 1) * 128][_PERM].T.reshape(kb_n, 128, 128)
        vrows = wv[g * 128:(g + 1) * 128].T.reshape(kb_n, 128, 128)
        wqk[p_, :, :, hh * 128:(hh + 1) * 128] = qrows.astype(bf)
        wqk[p_, :, :, 256 + hh * 128:256 + (hh + 1) * 128] = krows.astype(bf)
        wvs[p_, :, :, hh * 128:(hh + 1) * 128] = vrows.astype(bf)
        wos[:, lh, :] = wo[:, g * 128:(g + 1) * 128].T.astype(bf)
    # rope tables transposed; pair i lives at partitions (i, 64+i)
    cosT = np.ascontiguousarray(cos[:s].T).astype(np.float32)
    sinT = np.ascontiguousarray(sin[:s].T).astype(np.float32)
    # adapter projections on host (tiny): a_k^T (perm rows) and tanh(g)*a_v
    hsl = slice(g0 * 128, (g0 + hpc) * 128)
    ak = adapter[bi] @ wk[hsl].T          # [AL, hpc*128]
    av = adapter[bi] @ wv[hsl].T
    akT = np.empty((128, hpc, AL), bf)
    avs = np.empty((AL, hpc, 128), bf)
    for lh in range(hpc):
        akT[:, lh, :] = ak[:, lh * 128:(lh + 1) * 128].T[_PERM].astype(bf)
        g_ = math.tanh(float(gate[0, g0 + lh, 0, 0]))
        avs[:, lh, :] = (av[:, lh * 128:(lh + 1) * 128] * g_).astype(bf)
    return {
        "xT": xT, "wqk": wqk, "wv": wvs, "wo": wos,
        "cosT": cosT, "sinT": sinT, "akT": akT, "av": avs,
    }


def _numpy_reference(x, mask, cos, sin, wq, wk, wv, wo, gate, adapter):
    """Fallback (and general-mask) path in fp32 numpy."""
    bsz, seqlen, dm = x.shape
    h = wq.shape[0] // HD
    sc = 1.0 / math.sqrt(HD)

    def rope(t):
        tr = t.reshape(*t.shape[:-1], HD // 2, 2)
        t0, t1 = tr[..., 0], tr[..., 1]
        c = cos[None, :, None, :]
        s_ = sin[None, :, None, :]
        r0 = t0 * c - t1 * s_
        r1 = t0 * s_ + t1 * c
        return np.stack([r0, r1], axis=-1).reshape(t.shape)

    xq = (x @ wq.T).reshape(bsz, seqlen, h, HD)
    xk = (x @ wk.T).reshape(bsz, seqlen, h, HD)
    xv = (x @ wv.T).reshape(bsz, seqlen, h, HD)
    q = rope(xq).transpose(0, 2, 1, 3)
    k = rope(xk).transpose(0, 2, 1, 3)
    v = xv.transpose(0, 2, 1, 3)
    sc_ = np.einsum("bhqd,bhkd->bhqk", q, k) * sc + mask
    sc_ = sc_ - sc_.max(-1, keepdims=True)
    e = np.exp(sc_)
    p = e / e.sum(-1, keepdims=True)
    out = np.einsum("bhqk,bhkd->bhqd", p, v)
    al = adapter.shape[1]
    av = (adapter @ wv.T).reshape(bsz, al, h, HD).transpose(0, 2, 1, 3)
    ak = (adapter @ wk.T).reshape(bsz, al, h, HD).transpose(0, 2, 1, 3)
    asc = np.einsum("bhqd,bhkd->bhqk", q, ak) * sc
    asc = asc - asc.max(-1, keepdims=True)
    ae = np.exp(asc)
    ap = np.tanh(gate) * ae / ae.sum(-1, keepdims=True)
    out = out + np.einsum("bhqk,bhkd->bhqd", ap, av)
    out = out.transpose(0, 2, 1, 3).reshape(bsz, seqlen, -1)
    return (out @ wo.T).astype(np.float32)


_NC_CACHE = {}


def kernel(x, mask, cos, sin, wq, wk, wv, wo, gate, adapter, start_pos):
    x = np.asarray(x, np.float32)
    mask = np.asarray(mask, np.float32)
    cos = np.asarray(cos, np.float32)
    sin = np.asarray(sin, np.float32)
    wq = np.asarray(wq, np.float32)
    wk = np.asarray(wk, np.float32)
    wv = np.asarray(wv, np.float32)
    wo = np.asarray(wo, np.float32)
    gate = np.asarray(gate, np.float32)
    adapter = np.asarray(adapter, np.float32)

    causal = np.triu(np.full((S, S), -1e9, np.float32), 1)[None, None]
    if (x.shape != (B, S, D) or int(start_pos) != 0
            or not np.array_equal(mask, causal)):
        return _numpy_reference(x, mask, cos, sin, wq, wk, wv, wo, gate, adapter)

    if "nc" not in _NC_CACHE:
        _NC_CACHE["nc"] = build_nc()
    nc = _NC_CACHE["nc"]
    xT_cache = {}
    in_maps = [
        _host_inputs(c, x, cos, sin, wq, wk, wv, wo, gate, adapter,
                     xT_cache=xT_cache)
        for c in range(NCORES)
    ]
    res = run_bass_kernel_spmd(nc, in_maps, list(range(NCORES)))
    gpb = NCORES // B
    out = np.empty((B, S, D), np.float64)
    for bi in range(B):
        acc = res.results[bi * gpb]["out"].astype(np.float64)
        for c in range(bi * gpb + 1, (bi + 1) * gpb):
            acc += res.results[c]["out"]
        out[bi] = acc
    return out.astype(np.float32)
